# revision 32
# baseline (speedup 1.0000x reference)
"""IrregularRNN (exact LTC cell) Trainium2 Bass kernel.

Strategy: tensor-parallel split of the 2U=2048 pre-activation columns
across 8 cores. Core k computes pre columns {f: [k*128,(k+1)*128),
a: [U+k*128, U+(k+1)*128)} for the FULL batch B=128 (full PE
utilization), updates h columns [k*128,(k+1)*128), transposes its
h'-slice on the PE, and AllGathers the transposed slices so every core
has the full h^T (as 8 ready-to-use lhsT K-chunks) for the next step's
h @ Wh matmul.  The x_t @ Wx part + bias only depend on inputs, so
those matmuls are issued ahead and hide inside the AllGather wait.

Measured on the target fabric the AllGather itself costs ~10.5 us per
step regardless of payload (pure latency), so the device loop is
organized to keep everything else off the critical path:
 - sigmoid computed via tanh and folded into the decay algebra so the
   scalar engine only ever uses {Tanh, Exp, Copy, Identity} from one
   activation-table set -> no per-step LoadActFuncSet (1.3 us each).
 - pre split into separate f/a PSUM tiles; the f-part matmuls complete
   first so the decay chain overlaps the a-part matmuls, and the cell
   update uses h' = h*dcy + a*(1-dcy) with h*dcy and (1-dcy) computed
   before `a` is ready -> only 2 elementwise ops serialize after `a`.
 - gathered h^T landed as 8 chunk DMAs alternating between the SP and
   Activation DMA queues; the first chunk's matmul starts while later
   chunks are still in flight.
 - x_t prefetched two 8-step groups ahead, ys written back once per
   8-step group on the Activation queue.
 - optional warm-keeper matmuls during the AllGather wait keep the PE
   HAM clock-gate at full speed so the critical h-matmuls run at
   2.4 GHz instead of 1.2 GHz.

All layout transforms (transposes, weight slicing, broadcast of tau)
are done host-side in numpy; the device loop is 256 fully-unrolled
steps.
"""

import sys

sys.path.insert(0, "/opt/trn_rl_repo")

import numpy as np

B, T, D, U = 128, 256, 256, 1024
NC = 8
SL = U // NC          # h columns per core (128)
PW = 2 * SL           # pre-activation columns per core (256)
DK = D // 128         # K-chunks for x part (2)
UK = U // 128         # K-chunks for h part (8)
GR = 8                # steps per DMA group (xt prefetch / ys writeback)

_CACHE: dict = {}


def _build(n_steps: int, use_collective: bool = True, repeat: int = 1,
           warm_mms: int = 0, drop_transpose: bool = False,
           drop_acts: bool = False, hmm_count: int = UK,
           const_cc_in: bool = False, warm_paced: int = 0,
           dpool_bufs: int = 4):
    """Build + bacc-compile the SPMD Bass module for n_steps timesteps.

    repeat>1 is a timing-only mode: the T-loop body runs repeat times over
    the same inputs/outputs (numerically wrong; isolates on-device exec
    time from harness data-shipping via wall(2x) - wall(1x))."""
    import concourse.bacc as bacc
    import concourse.tile as tile
    from concourse import mybir

    AF = mybir.ActivationFunctionType
    f32 = mybir.dt.float32
    assert n_steps % GR == 0
    NG = n_steps // GR

    nc = bacc.Bacc(
        "TRN2",
        target_bir_lowering=False,
        debug=False,
        enable_asserts=False,
        num_devices=NC,
    )

    # --- kernel I/O ---------------------------------------------------
    xT = nc.dram_tensor("xT", [NG, GR, DK, 128, B], f32, kind="ExternalInput")
    wx_sl = nc.dram_tensor("wx_sl", [DK, 128, PW], f32, kind="ExternalInput")
    wh_sl = nc.dram_tensor("wh_sl", [UK, 128, PW], f32, kind="ExternalInput")
    b_sl = nc.dram_tensor("b_sl", [1, PW], f32, kind="ExternalInput")
    # tau2 = 2*softplus(w_tau)+1, broadcast to [B, SL]
    tau_b = nc.dram_tensor("tau_b", [B, SL], f32, kind="ExternalInput")
    # ndt = -time_steps/2  [B, T]
    ndt = nc.dram_tensor("ndt", [B, n_steps], f32, kind="ExternalInput")
    h0T = nc.dram_tensor("h0T", [UK, 128, B], f32, kind="ExternalInput")
    h0_sl = nc.dram_tensor("h0_sl", [B, SL], f32, kind="ExternalInput")
    ones = nc.dram_tensor("ones", [1, 128], f32, kind="ExternalInput")
    ident = nc.dram_tensor("ident", [128, 128], f32, kind="ExternalInput")
    ys_sl = nc.dram_tensor("ys_sl", [NG, B, GR, SL], f32, kind="ExternalOutput")

    RG = [list(range(NC))]

    with tile.TileContext(nc) as tc:
        with (
            tc.tile_pool(name="const", bufs=1) as cpool,
            tc.tile_pool(name="xin", bufs=3) as xpool,
            tc.tile_pool(name="hT", bufs=2) as hTpool,
            tc.tile_pool(name="act", bufs=3) as apool,
            tc.tile_pool(name="ys", bufs=2) as ypool,
            tc.tile_pool(name="pref", bufs=2, space="PSUM") as pfpool,
            tc.tile_pool(name="prea", bufs=2, space="PSUM") as papool,
            tc.tile_pool(name="trp", bufs=2, space="PSUM") as trpool,
            tc.tile_pool(name="warm", bufs=1, space="PSUM") as wpool,
            tc.tile_pool(name="agio", bufs=dpool_bufs, space="DRAM") as dpool,
            tc.tile_pool(name="dconst", bufs=1, space="DRAM") as dcpool,
        ):
            # --- constants, loaded once -------------------------------
            wx_sb = cpool.tile([128, DK, PW], f32, name="wx_sb")
            nc.sync.dma_start(out=wx_sb[:], in_=wx_sl.ap().rearrange("c p n -> p c n"))
            wh_sb = cpool.tile([128, UK, PW], f32, name="wh_sb")
            nc.sync.dma_start(out=wh_sb[:], in_=wh_sl.ap().rearrange("c p n -> p c n"))
            b_sb = cpool.tile([1, PW], f32, name="b_sb")
            nc.sync.dma_start(out=b_sb[:], in_=b_sl[:])
            ones_sb = cpool.tile([1, 128], f32, name="ones_sb")
            nc.sync.dma_start(out=ones_sb[:], in_=ones[:])
            tau_sb = cpool.tile([B, SL], f32, name="tau_sb")
            nc.sync.dma_start(out=tau_sb[:], in_=tau_b[:])
            ndt_sb = cpool.tile([B, n_steps], f32, name="ndt_sb")
            nc.sync.dma_start(out=ndt_sb[:], in_=ndt[:])
            ident_sb = cpool.tile([128, 128], f32, name="ident_sb")
            nc.sync.dma_start(out=ident_sb[:], in_=ident[:])

            # initial state
            h_first = cpool.tile([B, SL], f32, name="h_first")
            nc.sync.dma_start(out=h_first[:], in_=h0_sl[:])
            h_prev = h_first[:]

            cc_const = None
            if const_cc_in:
                # never-rewritten DRAM staging tile for the CC-timing bisect
                cc_const = dcpool.tile([128, B], f32, name="cc_const")
                tmp0 = cpool.tile([128, B], f32, name="ccc_sb")
                nc.sync.dma_start(out=tmp0[:], in_=h0T.ap()[0])
                nc.sync.dma_start(out=cc_const[:], in_=tmp0[:])

            warm_ps = (
                wpool.tile([B, 512], f32, name="warm_ps")
                if (warm_mms or warm_paced)
                else None
            )
            junk_sb = junk_dram = None
            if warm_paced:
                # HAM pacemaker: a DMA<->matmul dependency chain that drips a
                # junk matmul onto the PE every couple of microseconds during
                # the AllGather wait, so the PE clock gate never sees an idle
                # MID window and the critical h-matmuls run at 2.4 GHz.
                junk_sb = cpool.tile([128, DK, PW], f32, name="junk_sb")
                nc.sync.dma_start(
                    out=junk_sb[:],
                    in_=wx_sl.ap().rearrange("c p n -> p c n"),
                )
                junk_dram = dcpool.tile([128, DK, PW], f32, name="junk_dram")
                nc.sync.dma_start(out=junk_dram[:], in_=junk_sb[:])

            def land_hT(src_dram, engines):
                """DMA the gathered h^T [UK*128, B] into SBUF chunk by chunk;
                returns chunk_ap(j) accessor for the j-th [128, B] K-chunk."""
                tiles = []
                for j in range(UK):
                    tj = hTpool.tile([128, B], f32, name=f"hT{j}")
                    engines[j % len(engines)].dma_start(
                        out=tj[:], in_=src_dram[j * 128 : (j + 1) * 128, :]
                    )
                    tiles.append(tj)
                return lambda j: tiles[j][:]

            hT_ap = land_hT(
                h0T.ap().rearrange("c p b -> (c p) b"), (nc.sync, nc.scalar)
            )

            # --- the recurrence ---------------------------------------
            n_total = n_steps * repeat

            def prefetch_group(gv):
                """Prefetch the xT group for virtual group gv (one DMA)."""
                g = (gv * GR) % n_steps // GR
                tile_ = xpool.tile([128, GR, DK, B], f32, name="xt_g")
                nc.sync.dma_start(
                    out=tile_[:], in_=xT.ap()[g].rearrange("g c p b -> p g c b")
                )
                return tile_

            xt_tiles = {}
            NGV = (n_total + GR - 1) // GR
            for pg in range(min(2, NGV)):
                xt_tiles[pg] = prefetch_group(pg)

            for tv in range(n_total):
                t = tv % n_steps
                i = tv % GR
                gv = tv // GR
                if i == 0:
                    xt_g = xt_tiles.pop(gv)
                    if gv + 2 < NGV:
                        xt_tiles[gv + 2] = prefetch_group(gv + 2)
                    # group output buffer; slices double as the h state
                    ys_buf = ypool.tile([B, GR, SL], f32, name="ys_buf")

                pre_f = pfpool.tile([B, SL], f32, name="pre_f")
                pre_a = papool.tile([B, SL], f32, name="pre_a")
                # bias + x part: no dependency on h -> runs during the
                # previous step's AllGather wait.
                nc.tensor.matmul(
                    pre_f[:], ones_sb[:], b_sb[:, 0:SL], start=True, stop=False
                )
                nc.tensor.matmul(
                    pre_a[:], ones_sb[:], b_sb[:, SL:PW], start=True, stop=False
                )
                for c in range(DK):
                    nc.tensor.matmul(
                        pre_f[:], xt_g[:, i, c, :], wx_sb[:, c, 0:SL],
                        start=False, stop=False,
                    )
                    nc.tensor.matmul(
                        pre_a[:], xt_g[:, i, c, :], wx_sb[:, c, SL:PW],
                        start=False, stop=False,
                    )
                # h part: f-half first (so the decay chain can start while
                # the a-half matmuls still run), then a-half.
                for j in range(hmm_count):
                    nc.tensor.matmul(
                        pre_f[:],
                        hT_ap(j if use_collective else 0),
                        wh_sb[:, j, 0:SL],
                        start=False,
                        stop=(j == hmm_count - 1),
                    )
                for j in range(hmm_count):
                    nc.tensor.matmul(
                        pre_a[:],
                        hT_ap(j if use_collective else 0),
                        wh_sb[:, j, SL:PW],
                        start=False,
                        stop=(j == hmm_count - 1),
                    )

                h_new = ys_buf[:, i, :]
                if drop_acts:
                    # timing bisect: skip the whole elementwise chain
                    nc.vector.tensor_copy(h_new, pre_f[:])
                else:
                    # decay chain (overlaps the a-half matmuls):
                    # sigmoid via tanh so every activation lives in one
                    # act-table set -> no LoadActFuncSet per step:
                    # exp(-dt*(tau+sigm(p))) = exp(-dt/2*(tanh(p/2)+2*tau+1))
                    f = apool.tile([B, SL], f32, name="f")
                    nc.scalar.activation(f[:], pre_f[:], AF.Tanh, scale=0.5)
                    g = apool.tile([B, SL], f32, name="g")
                    nc.vector.tensor_add(g[:], f[:], tau_sb[:])
                    dcy = apool.tile([B, SL], f32, name="dcy")
                    nc.scalar.activation(
                        dcy[:], g[:], AF.Exp, scale=ndt_sb[:, t : t + 1]
                    )
                    # h' = h*dcy + a*(1-dcy); h*dcy, (1-dcy) ready before a
                    hdc = apool.tile([B, SL], f32, name="hdc")
                    nc.vector.tensor_mul(hdc[:], h_prev, dcy[:])
                    om = apool.tile([B, SL], f32, name="om")
                    nc.scalar.activation(
                        om[:], dcy[:], AF.Copy, bias=1.0, scale=-1.0
                    )

                    a = apool.tile([B, SL], f32, name="a")
                    nc.scalar.activation(a[:], pre_a[:], AF.Tanh)
                    aom = apool.tile([B, SL], f32, name="aom")
                    nc.vector.tensor_mul(aom[:], a[:], om[:])
                    nc.vector.tensor_add(h_new, aom[:], hdc[:])

                if i == GR - 1:
                    nc.scalar.dma_start(out=ys_sl[gv % NG], in_=ys_buf[:])

                if tv == n_total - 1:
                    break

                # h'^T slice for the next step's matmul
                ag_in = dpool.tile([128, B], f32, name="ag_in")
                if drop_transpose:
                    # timing bisect: ship h_new untransposed (wrong values,
                    # same bytes/dependency structure minus transpose+copy)
                    nc.sync.dma_start(out=ag_in[:], in_=h_new)
                else:
                    trp = trpool.tile([128, B], f32, name="trp")
                    nc.tensor.transpose(trp[:], h_new, ident_sb[:])
                    trs = apool.tile([128, B], f32, name="trs")
                    nc.vector.tensor_copy(trs[:], trp[:])
                    nc.sync.dma_start(out=ag_in[:], in_=trs[:])
                if use_collective:
                    ag_out = dpool.tile(
                        [UK * 128, B], f32, name="ag_out", addr_space="Shared"
                    )
                    # const_cc_in: timing bisect — CC reads a never-rewritten
                    # DRAM tile, so it does not depend on this step's compute
                    cc_src = cc_const[:] if const_cc_in else ag_in[:]
                    nc.gpsimd.collective_compute(
                        "AllGather",
                        mybir.AluOpType.bypass,
                        replica_groups=RG,
                        ins=[cc_src.opt()],
                        outs=[ag_out[:].opt()],
                    )
                    src = ag_out
                else:
                    src = ag_in

                # warm-keepers: junk back-to-back matmuls with no deps that
                # fill the AllGather wait so the PE HAM gate stays at 2.4 GHz
                for w in range(warm_mms):
                    j2 = (w % (UK // 2)) * 2
                    nc.tensor.matmul(
                        warm_ps[:], wh_sb[:, w % UK, 0:128],
                        wh_sb[:, j2 : j2 + 2, :],
                        start=True, stop=True, skip_group_check=True,
                    )
                # paced warm-keepers: each junk matmul waits on a gpsimd DMA
                # rewrite of its input (RAW), and the next DMA waits on the
                # matmul (WAR) -> one ~500 ns matmul lands on the PE every
                # ~2 us through the AllGather window.
                for w in range(warm_paced):
                    nc.tensor.matmul(
                        warm_ps[:], junk_sb[:, 0, 0:128], junk_sb[:],
                        start=True, stop=True, skip_group_check=True,
                    )
                    nc.gpsimd.dma_start(out=junk_sb[:], in_=junk_dram[:])

                if use_collective:
                    hT_ap = land_hT(src[:], (nc.sync, nc.scalar))
                else:
                    # timing bisect: keep the serial dependence on ag_in but
                    # load only one chunk (matmuls reuse chunk 0)
                    t0_ = hTpool.tile([128, B], f32, name="hT0")
                    nc.sync.dma_start(out=t0_[:], in_=src[:])
                    hT_ap = (lambda tt: lambda j: tt[:])(t0_)
                h_prev = h_new

    nc.compile()
    return nc


def _prep_inputs(features, time_steps, Wx, Wh, b, w_tau, h0, n_steps):
    """Host-side sharding + layout transforms -> per-core in_maps."""
    f32 = np.float32
    features = np.asarray(features, dtype=f32)
    time_steps = np.asarray(time_steps, dtype=f32)
    Wx = np.asarray(Wx, dtype=f32)
    Wh = np.asarray(Wh, dtype=f32)
    b = np.asarray(b, dtype=f32)
    w_tau = np.asarray(w_tau, dtype=f32)
    h0 = np.asarray(h0, dtype=f32)

    # tau2 = 2*softplus(w_tau) + 1, fp32 (see tanh-sigmoid identity in _build)
    tau = (2.0 * np.log1p(np.exp(w_tau)) + 1.0).astype(f32)

    xT = np.ascontiguousarray(features.transpose(1, 2, 0)).reshape(
        n_steps // GR, GR, DK, 128, B
    )
    ndt = np.ascontiguousarray(-0.5 * time_steps)                # [B, T]
    h0T = np.ascontiguousarray(h0.T).reshape(UK, 128, B)
    ones = np.ones((1, 128), dtype=f32)
    ident = np.eye(128, dtype=f32)

    in_maps = []
    for k in range(NC):
        cols = np.concatenate(
            [np.arange(k * SL, (k + 1) * SL), U + np.arange(k * SL, (k + 1) * SL)]
        )
        in_maps.append(
            {
                "xT": xT,
                "wx_sl": np.ascontiguousarray(Wx[:, cols]).reshape(DK, 128, PW),
                "wh_sl": np.ascontiguousarray(Wh[:, cols]).reshape(UK, 128, PW),
                "b_sl": np.ascontiguousarray(b[cols]).reshape(1, PW),
                "tau_b": np.ascontiguousarray(
                    np.broadcast_to(tau[k * SL : (k + 1) * SL], (B, SL))
                ),
                "ndt": ndt,
                "h0T": h0T,
                "h0_sl": np.ascontiguousarray(h0[:, k * SL : (k + 1) * SL]),
                "ones": ones,
                "ident": ident,
            }
        )
    return in_maps


def _assemble(results):
    """[T/GR, B, GR, SL] slices per core -> [B, T, U] full output."""
    ys = np.concatenate([r["ys_sl"] for r in results], axis=3)  # [NG, B, GR, U]
    ys = ys.transpose(1, 0, 2, 3).reshape(B, -1, U)
    return np.ascontiguousarray(ys)


def kernel(features, time_steps, Wx, Wh, b, w_tau, h0, _trace=False):
    from concourse import bass_utils

    n_steps = features.shape[1]
    if n_steps not in _CACHE:
        _CACHE[n_steps] = _build(n_steps)
    nc = _CACHE[n_steps]

    in_maps = _prep_inputs(features, time_steps, Wx, Wh, b, w_tau, h0, n_steps)
    try:
        res = bass_utils.run_bass_kernel_spmd(
            nc, in_maps, core_ids=list(range(NC)), trace=_trace
        )
    except ModuleNotFoundError:
        # no NTFF profiling hook in this container — run untraced
        res = bass_utils.run_bass_kernel_spmd(
            nc, in_maps, core_ids=list(range(NC)), trace=False
        )
    out = _assemble(res.results)
    if _trace:
        return out, res
    return out


if __name__ == "__main__":
    # smoke test with random data
    rng = np.random.default_rng(0)
    feats = rng.standard_normal((B, T, D), dtype=np.float32)
    ts = rng.random((B, T), dtype=np.float32)
    Wx = rng.standard_normal((D, 2 * U), dtype=np.float32) / np.sqrt(D)
    Wh = rng.standard_normal((U, 2 * U), dtype=np.float32) / np.sqrt(U)
    b = np.zeros((2 * U,), dtype=np.float32)
    w_tau = rng.random((U,), dtype=np.float32)
    h0 = np.zeros((B, U), dtype=np.float32)
    out = kernel(feats, ts, Wx, Wh, b, w_tau, h0)
    print("output", out.shape, out.dtype)


# revision 33
# speedup vs baseline: 1.3337x; 1.3337x over previous
"""IrregularRNN (exact LTC cell) Trainium2 Bass kernel.

Strategy: tensor-parallel split of the 2U=2048 pre-activation columns
across 8 cores. Core k computes pre columns {f: [k*128,(k+1)*128),
a: [U+k*128, U+(k+1)*128)} for the FULL batch B=128 (full PE
utilization), updates h columns [k*128,(k+1)*128), transposes its
h'-slice on the PE, and AllGathers the transposed slices so every core
has the full h^T (as 8 ready-to-use lhsT K-chunks) for the next step's
h @ Wh matmul.  The x_t @ Wx part + bias only depend on inputs, so
those matmuls are issued ahead and hide inside the AllGather wait.

Measured on the target fabric the AllGather itself costs ~10.5 us per
step regardless of payload (pure latency), so the device loop is
organized to keep everything else off the critical path:
 - sigmoid computed via tanh and folded into the decay algebra so the
   scalar engine only ever uses {Tanh, Exp, Copy, Identity} from one
   activation-table set -> no per-step LoadActFuncSet (1.3 us each).
 - pre split into separate f/a PSUM tiles; the f-part matmuls complete
   first so the decay chain overlaps the a-part matmuls, and the cell
   update uses h' = h*dcy + a*(1-dcy) with h*dcy and (1-dcy) computed
   before `a` is ready -> only 2 elementwise ops serialize after `a`.
 - gathered h^T landed as 8 chunk DMAs alternating between the SP and
   Activation DMA queues; the first chunk's matmul starts while later
   chunks are still in flight.
 - x_t prefetched two 8-step groups ahead, ys written back once per
   8-step group on the Activation queue.
 - optional warm-keeper matmuls during the AllGather wait keep the PE
   HAM clock-gate at full speed so the critical h-matmuls run at
   2.4 GHz instead of 1.2 GHz.

All layout transforms (transposes, weight slicing, broadcast of tau)
are done host-side in numpy; the device loop is 256 fully-unrolled
steps.
"""

import sys

sys.path.insert(0, "/opt/trn_rl_repo")

import numpy as np

B, T, D, U = 128, 256, 256, 1024
NC = 8
SL = U // NC          # h columns per core (128)
PW = 2 * SL           # pre-activation columns per core (256)
DK = D // 128         # K-chunks for x part (2)
UK = U // 128         # K-chunks for h part (8)
GR = 8                # steps per DMA group (xt prefetch / ys writeback)

_CACHE: dict = {}


def _build(n_steps: int, use_collective: bool = True, repeat: int = 1,
           warm_mms: int = 0, drop_transpose: bool = False,
           drop_acts: bool = False, hmm_count: int = UK,
           const_cc_in: bool = False, warm_paced: int = 0,
           dpool_bufs: int = 4):
    """Build + bacc-compile the SPMD Bass module for n_steps timesteps.

    repeat>1 is a timing-only mode: the T-loop body runs repeat times over
    the same inputs/outputs (numerically wrong; isolates on-device exec
    time from harness data-shipping via wall(2x) - wall(1x))."""
    import concourse.bacc as bacc
    import concourse.tile as tile
    from concourse import mybir

    AF = mybir.ActivationFunctionType
    f32 = mybir.dt.float32
    assert n_steps % GR == 0
    NG = n_steps // GR

    nc = bacc.Bacc(
        "TRN2",
        target_bir_lowering=False,
        debug=False,
        enable_asserts=False,
        num_devices=NC,
    )

    # --- kernel I/O ---------------------------------------------------
    xT = nc.dram_tensor("xT", [NG, GR, DK, 128, B], f32, kind="ExternalInput")
    wx_sl = nc.dram_tensor("wx_sl", [DK, 128, PW], f32, kind="ExternalInput")
    wh_sl = nc.dram_tensor("wh_sl", [UK, 128, PW], f32, kind="ExternalInput")
    b_sl = nc.dram_tensor("b_sl", [1, PW], f32, kind="ExternalInput")
    # tau2 = 2*softplus(w_tau)+1, broadcast to [B, SL]
    tau_b = nc.dram_tensor("tau_b", [B, SL], f32, kind="ExternalInput")
    # ndt = -time_steps/2  [B, T]
    ndt = nc.dram_tensor("ndt", [B, n_steps], f32, kind="ExternalInput")
    h0T = nc.dram_tensor("h0T", [UK, 128, B], f32, kind="ExternalInput")
    h0_sl = nc.dram_tensor("h0_sl", [B, SL], f32, kind="ExternalInput")
    ones = nc.dram_tensor("ones", [1, 128], f32, kind="ExternalInput")
    ident = nc.dram_tensor("ident", [128, 128], f32, kind="ExternalInput")
    ys_sl = nc.dram_tensor("ys_sl", [NG, B, GR, SL], f32, kind="ExternalOutput")

    RG = [list(range(NC))]

    with tile.TileContext(nc) as tc:
        with (
            tc.tile_pool(name="const", bufs=1) as cpool,
            tc.tile_pool(name="xin", bufs=3) as xpool,
            tc.tile_pool(name="hT", bufs=2) as hTpool,
            tc.tile_pool(name="act", bufs=3) as apool,
            tc.tile_pool(name="ys", bufs=2) as ypool,
            tc.tile_pool(name="pref", bufs=2, space="PSUM") as pfpool,
            tc.tile_pool(name="prea", bufs=2, space="PSUM") as papool,
            tc.tile_pool(name="trp", bufs=2, space="PSUM") as trpool,
            tc.tile_pool(name="warm", bufs=1, space="PSUM") as wpool,
            tc.tile_pool(name="agio", bufs=dpool_bufs, space="DRAM") as dpool,
            tc.tile_pool(name="dconst", bufs=1, space="DRAM") as dcpool,
        ):
            # --- constants, loaded once -------------------------------
            wx_sb = cpool.tile([128, DK, PW], f32, name="wx_sb")
            nc.sync.dma_start(out=wx_sb[:], in_=wx_sl.ap().rearrange("c p n -> p c n"))
            wh_sb = cpool.tile([128, UK, PW], f32, name="wh_sb")
            nc.sync.dma_start(out=wh_sb[:], in_=wh_sl.ap().rearrange("c p n -> p c n"))
            b_sb = cpool.tile([1, PW], f32, name="b_sb")
            nc.sync.dma_start(out=b_sb[:], in_=b_sl[:])
            ones_sb = cpool.tile([1, 128], f32, name="ones_sb")
            nc.sync.dma_start(out=ones_sb[:], in_=ones[:])
            tau_sb = cpool.tile([B, SL], f32, name="tau_sb")
            nc.sync.dma_start(out=tau_sb[:], in_=tau_b[:])
            ndt_sb = cpool.tile([B, n_steps], f32, name="ndt_sb")
            nc.sync.dma_start(out=ndt_sb[:], in_=ndt[:])
            ident_sb = cpool.tile([128, 128], f32, name="ident_sb")
            nc.sync.dma_start(out=ident_sb[:], in_=ident[:])

            # initial state
            h_first = cpool.tile([B, SL], f32, name="h_first")
            nc.sync.dma_start(out=h_first[:], in_=h0_sl[:])
            h_prev = h_first[:]

            cc_const = None
            if const_cc_in:
                # never-rewritten DRAM staging tile for the CC-timing bisect
                cc_const = dcpool.tile([128, B], f32, name="cc_const")
                tmp0 = cpool.tile([128, B], f32, name="ccc_sb")
                nc.sync.dma_start(out=tmp0[:], in_=h0T.ap()[0])
                nc.sync.dma_start(out=cc_const[:], in_=tmp0[:])

            warm_ps = (
                wpool.tile([B, 512], f32, name="warm_ps")
                if (warm_mms or warm_paced)
                else None
            )
            junk_sb = junk_dram = None
            if warm_paced:
                # HAM pacemaker: a DMA<->matmul dependency chain that drips a
                # junk matmul onto the PE every couple of microseconds during
                # the AllGather wait, so the PE clock gate never sees an idle
                # MID window and the critical h-matmuls run at 2.4 GHz.
                junk_sb = cpool.tile([128, DK, PW], f32, name="junk_sb")
                nc.sync.dma_start(
                    out=junk_sb[:],
                    in_=wx_sl.ap().rearrange("c p n -> p c n"),
                )
                junk_dram = dcpool.tile([128, DK, PW], f32, name="junk_dram")
                nc.sync.dma_start(out=junk_dram[:], in_=junk_sb[:])

            def land_hT(src_dram, engines):
                """DMA the gathered h^T [UK*128, B] into SBUF chunk by chunk;
                returns chunk_ap(j) accessor for the j-th [128, B] K-chunk."""
                tiles = []
                for j in range(UK):
                    tj = hTpool.tile([128, B], f32, name=f"hT{j}")
                    engines[j % len(engines)].dma_start(
                        out=tj[:], in_=src_dram[j * 128 : (j + 1) * 128, :]
                    )
                    tiles.append(tj)
                return lambda j: tiles[j][:]

            hT_ap = land_hT(
                h0T.ap().rearrange("c p b -> (c p) b"), (nc.sync, nc.scalar)
            )

            # --- the recurrence ---------------------------------------
            n_total = n_steps * repeat

            def prefetch_group(gv):
                """Prefetch the xT group for virtual group gv (one DMA)."""
                g = (gv * GR) % n_steps // GR
                tile_ = xpool.tile([128, GR, DK, B], f32, name="xt_g")
                nc.gpsimd.dma_start(
                    out=tile_[:], in_=xT.ap()[g].rearrange("g c p b -> p g c b")
                )
                return tile_

            xt_tiles = {}
            NGV = (n_total + GR - 1) // GR
            for pg in range(min(2, NGV)):
                xt_tiles[pg] = prefetch_group(pg)

            for tv in range(n_total):
                t = tv % n_steps
                i = tv % GR
                gv = tv // GR
                if i == 0:
                    xt_g = xt_tiles.pop(gv)
                    if gv + 2 < NGV:
                        xt_tiles[gv + 2] = prefetch_group(gv + 2)
                    # group output buffer; slices double as the h state
                    ys_buf = ypool.tile([B, GR, SL], f32, name="ys_buf")

                pre_f = pfpool.tile([B, SL], f32, name="pre_f")
                pre_a = papool.tile([B, SL], f32, name="pre_a")
                # bias + x part: no dependency on h -> runs during the
                # previous step's AllGather wait.
                nc.tensor.matmul(
                    pre_f[:], ones_sb[:], b_sb[:, 0:SL], start=True, stop=False
                )
                nc.tensor.matmul(
                    pre_a[:], ones_sb[:], b_sb[:, SL:PW], start=True, stop=False
                )
                for c in range(DK):
                    nc.tensor.matmul(
                        pre_f[:], xt_g[:, i, c, :], wx_sb[:, c, 0:SL],
                        start=False, stop=False,
                    )
                    nc.tensor.matmul(
                        pre_a[:], xt_g[:, i, c, :], wx_sb[:, c, SL:PW],
                        start=False, stop=False,
                    )
                # h part: f-half first (so the decay chain can start while
                # the a-half matmuls still run), then a-half.
                for j in range(hmm_count):
                    nc.tensor.matmul(
                        pre_f[:],
                        hT_ap(j if use_collective else 0),
                        wh_sb[:, j, 0:SL],
                        start=False,
                        stop=(j == hmm_count - 1),
                    )
                for j in range(hmm_count):
                    nc.tensor.matmul(
                        pre_a[:],
                        hT_ap(j if use_collective else 0),
                        wh_sb[:, j, SL:PW],
                        start=False,
                        stop=(j == hmm_count - 1),
                    )

                h_new = ys_buf[:, i, :]
                if drop_acts:
                    # timing bisect: skip the whole elementwise chain
                    nc.vector.tensor_copy(h_new, pre_f[:])
                else:
                    # decay chain (overlaps the a-half matmuls):
                    # sigmoid via tanh so every activation lives in one
                    # act-table set -> no LoadActFuncSet per step:
                    # exp(-dt*(tau+sigm(p))) = exp(-dt/2*(tanh(p/2)+2*tau+1))
                    f = apool.tile([B, SL], f32, name="f")
                    nc.scalar.activation(f[:], pre_f[:], AF.Tanh, scale=0.5)
                    g = apool.tile([B, SL], f32, name="g")
                    nc.vector.tensor_add(g[:], f[:], tau_sb[:])
                    dcy = apool.tile([B, SL], f32, name="dcy")
                    nc.scalar.activation(
                        dcy[:], g[:], AF.Exp, scale=ndt_sb[:, t : t + 1]
                    )
                    # h' = h*dcy + a*(1-dcy); h*dcy, (1-dcy) ready before a
                    hdc = apool.tile([B, SL], f32, name="hdc")
                    nc.vector.tensor_mul(hdc[:], h_prev, dcy[:])
                    om = apool.tile([B, SL], f32, name="om")
                    nc.scalar.activation(
                        om[:], dcy[:], AF.Copy, bias=1.0, scale=-1.0
                    )

                    a = apool.tile([B, SL], f32, name="a")
                    nc.scalar.activation(a[:], pre_a[:], AF.Tanh)
                    aom = apool.tile([B, SL], f32, name="aom")
                    nc.vector.tensor_mul(aom[:], a[:], om[:])
                    nc.vector.tensor_add(h_new, aom[:], hdc[:])

                if i == GR - 1:
                    nc.gpsimd.dma_start(out=ys_sl[gv % NG], in_=ys_buf[:])

                if tv == n_total - 1:
                    break

                # h'^T slice for the next step's matmul
                ag_in = dpool.tile([128, B], f32, name="ag_in")
                if drop_transpose:
                    # timing bisect: ship h_new untransposed (wrong values,
                    # same bytes/dependency structure minus transpose+copy)
                    nc.sync.dma_start(out=ag_in[:], in_=h_new)
                else:
                    trp = trpool.tile([128, B], f32, name="trp")
                    nc.tensor.transpose(trp[:], h_new, ident_sb[:])
                    trs = apool.tile([128, B], f32, name="trs")
                    nc.vector.tensor_copy(trs[:], trp[:])
                    nc.sync.dma_start(out=ag_in[:], in_=trs[:])
                if use_collective:
                    ag_out = dpool.tile(
                        [UK * 128, B], f32, name="ag_out", addr_space="Shared"
                    )
                    # const_cc_in: timing bisect — CC reads a never-rewritten
                    # DRAM tile, so it does not depend on this step's compute
                    cc_src = cc_const[:] if const_cc_in else ag_in[:]
                    nc.gpsimd.collective_compute(
                        "AllGather",
                        mybir.AluOpType.bypass,
                        replica_groups=RG,
                        ins=[cc_src.opt()],
                        outs=[ag_out[:].opt()],
                    )
                    src = ag_out
                else:
                    src = ag_in

                # warm-keepers: junk back-to-back matmuls with no deps that
                # fill the AllGather wait so the PE HAM gate stays at 2.4 GHz
                for w in range(warm_mms):
                    j2 = (w % (UK // 2)) * 2
                    nc.tensor.matmul(
                        warm_ps[:], wh_sb[:, w % UK, 0:128],
                        wh_sb[:, j2 : j2 + 2, :],
                        start=True, stop=True, skip_group_check=True,
                    )
                # paced warm-keepers: each junk matmul waits on a gpsimd DMA
                # rewrite of its input (RAW), and the next DMA waits on the
                # matmul (WAR) -> one ~500 ns matmul lands on the PE every
                # ~2 us through the AllGather window.
                for w in range(warm_paced):
                    nc.tensor.matmul(
                        warm_ps[:], junk_sb[:, 0, 0:128], junk_sb[:],
                        start=True, stop=True, skip_group_check=True,
                    )
                    nc.gpsimd.dma_start(out=junk_sb[:], in_=junk_dram[:])

                if use_collective:
                    hT_ap = land_hT(src[:], (nc.sync, nc.scalar))
                else:
                    # timing bisect: keep the serial dependence on ag_in but
                    # load only one chunk (matmuls reuse chunk 0)
                    t0_ = hTpool.tile([128, B], f32, name="hT0")
                    nc.sync.dma_start(out=t0_[:], in_=src[:])
                    hT_ap = (lambda tt: lambda j: tt[:])(t0_)
                h_prev = h_new

    nc.compile()
    return nc


def _prep_inputs(features, time_steps, Wx, Wh, b, w_tau, h0, n_steps):
    """Host-side sharding + layout transforms -> per-core in_maps."""
    f32 = np.float32
    features = np.asarray(features, dtype=f32)
    time_steps = np.asarray(time_steps, dtype=f32)
    Wx = np.asarray(Wx, dtype=f32)
    Wh = np.asarray(Wh, dtype=f32)
    b = np.asarray(b, dtype=f32)
    w_tau = np.asarray(w_tau, dtype=f32)
    h0 = np.asarray(h0, dtype=f32)

    # tau2 = 2*softplus(w_tau) + 1, fp32 (see tanh-sigmoid identity in _build)
    tau = (2.0 * np.log1p(np.exp(w_tau)) + 1.0).astype(f32)

    xT = np.ascontiguousarray(features.transpose(1, 2, 0)).reshape(
        n_steps // GR, GR, DK, 128, B
    )
    ndt = np.ascontiguousarray(-0.5 * time_steps)                # [B, T]
    h0T = np.ascontiguousarray(h0.T).reshape(UK, 128, B)
    ones = np.ones((1, 128), dtype=f32)
    ident = np.eye(128, dtype=f32)

    in_maps = []
    for k in range(NC):
        cols = np.concatenate(
            [np.arange(k * SL, (k + 1) * SL), U + np.arange(k * SL, (k + 1) * SL)]
        )
        in_maps.append(
            {
                "xT": xT,
                "wx_sl": np.ascontiguousarray(Wx[:, cols]).reshape(DK, 128, PW),
                "wh_sl": np.ascontiguousarray(Wh[:, cols]).reshape(UK, 128, PW),
                "b_sl": np.ascontiguousarray(b[cols]).reshape(1, PW),
                "tau_b": np.ascontiguousarray(
                    np.broadcast_to(tau[k * SL : (k + 1) * SL], (B, SL))
                ),
                "ndt": ndt,
                "h0T": h0T,
                "h0_sl": np.ascontiguousarray(h0[:, k * SL : (k + 1) * SL]),
                "ones": ones,
                "ident": ident,
            }
        )
    return in_maps


def _assemble(results):
    """[T/GR, B, GR, SL] slices per core -> [B, T, U] full output."""
    ys = np.concatenate([r["ys_sl"] for r in results], axis=3)  # [NG, B, GR, U]
    ys = ys.transpose(1, 0, 2, 3).reshape(B, -1, U)
    return np.ascontiguousarray(ys)


def kernel(features, time_steps, Wx, Wh, b, w_tau, h0, _trace=False):
    from concourse import bass_utils

    n_steps = features.shape[1]
    if n_steps not in _CACHE:
        _CACHE[n_steps] = _build(n_steps)
    nc = _CACHE[n_steps]

    in_maps = _prep_inputs(features, time_steps, Wx, Wh, b, w_tau, h0, n_steps)
    try:
        res = bass_utils.run_bass_kernel_spmd(
            nc, in_maps, core_ids=list(range(NC)), trace=_trace
        )
    except ModuleNotFoundError:
        # no NTFF profiling hook in this container — run untraced
        res = bass_utils.run_bass_kernel_spmd(
            nc, in_maps, core_ids=list(range(NC)), trace=False
        )
    out = _assemble(res.results)
    if _trace:
        return out, res
    return out


if __name__ == "__main__":
    # smoke test with random data
    rng = np.random.default_rng(0)
    feats = rng.standard_normal((B, T, D), dtype=np.float32)
    ts = rng.random((B, T), dtype=np.float32)
    Wx = rng.standard_normal((D, 2 * U), dtype=np.float32) / np.sqrt(D)
    Wh = rng.standard_normal((U, 2 * U), dtype=np.float32) / np.sqrt(U)
    b = np.zeros((2 * U,), dtype=np.float32)
    w_tau = rng.random((U,), dtype=np.float32)
    h0 = np.zeros((B, U), dtype=np.float32)
    out = kernel(feats, ts, Wx, Wh, b, w_tau, h0)
    print("output", out.shape, out.dtype)
